# revision 44
# baseline (speedup 1.0000x reference)
"""GAT message-passing kernel for Trainium2 (8 NeuronCores, Bass/Tile).

Strategy v6 (edge-parallel, host-packed gather, gpsimd one-hot scatter):
y = elu(sum(xo[0] * xo[1:item_len], 1)) depends only on output rows
0..item_len-1, so only edges with dst < item_len contribute (~33.7k of
3.2M).  The host filters those edges, partitions them by dst block of
128 (core k owns dst rows [128k, 128k+128)), gathers x[src] rows and
packs them bf16 as [128 edges/tile | 256 feats + ones column] tiles.
Edges are arranged so every 8-partition half of a 16-row gpsimd group
shares one dst (~10% padding): ONE gpsimd indirect_copy (whose indices
are shared per 16-partition group) fetches a_d[dst] for both halves at
2 columns/tile, and a DVE select picks each row's half.

  per chunk of tiles (pipelined across engines):
    DVE:    a_s[:,t] = rowsum(xg_t * (W@att_src))        (stt accum)
    DVE:    v = a_s + ad; e = lrelu(v)                   (batched)
    Act:    p = exp(e)  -> bf16                          (batched)
    GpSimd: Sp[e, 128*i + dst] = p[e,i]  (local_scatter, one op/chunk;
            pad edges use index -1 and are dropped)
    PE:     acc += Sp_i^T @ [xg_i | 1]   (PSUM accumulate, bf16)
  acc is split in two PSUM halves so the first half's (u@W) epilogue
  overlaps the second half of the pipeline; u@W uses PE transposes.
  out = (u@W)/z + bias; xo = elu(out); y_k[j] = elu(dot(xo0, xo_j)).
  Every core also processes the dst==0 edges (block B, no one-hot:
  uB = p^T @ [xg|1], epilogue emitted early) so xo0 is ready before
  the main tail; Python concatenates the 8 y_k slices.
"""
import math

import numpy as np

P = 128
N_CORES = 8
NEG_SLOPE = 0.2
IND = 256
OUTD = 128

_CACHE = {}


def _chunk_sizes(T):
    # even-sized chunks; small first (fast pipeline start), small last
    sizes = [2]
    rem = T - 2
    while rem > 10:
        sizes.append(8)
        rem -= 8
    if rem > 2:
        sizes.append(rem - 2)
        sizes.append(2)
    elif rem > 0:
        sizes.append(rem)
    return sizes


def _build_program(T, TB, chunks):
    import concourse.bass as bass
    import concourse.bacc as bacc
    import concourse.tile as tile
    import concourse.mybir as mybir
    from contextlib import ExitStack

    f32 = mybir.dt.float32
    bf16 = mybir.dt.bfloat16
    i16 = mybir.dt.int16
    u16 = mybir.dt.uint16
    Alu = mybir.AluOpType
    Act = mybir.ActivationFunctionType
    W257 = IND + 1

    nc = bacc.Bacc(
        "TRN2", target_bir_lowering=False, debug=False, num_devices=N_CORES
    )
    C2 = (2 * T + 15) // 16
    # bf16 pack: [xb tiles | x0 | W halves | xg tiles]
    XGOFF = TB * W257 + IND + 2 * OUTD
    xg_in = nc.dram_tensor(
        "xg_in", [P, XGOFF + T * W257], bf16, kind="ExternalInput"
    ).ap()
    # scatter idx (chunk-local offsets, -1 pads) | gather idx (group-wrapped)
    dsti_in = nc.dram_tensor("dsti_in", [P, T], i16, kind="ExternalInput").ap()
    # gather idx | uint16 halfmask (rows 8..15 of each 16-group = 1)
    dstg_in = nc.dram_tensor(
        "dstg_in", [P, C2 + T], u16, kind="ExternalInput"
    ).ap()
    # f32 pack: ident(P) | WT(IND) | avs_b(P) | avd_b(P) | bias_b(OUTD) | maskB(TB)
    NF32 = P + IND + P + P + OUTD + TB
    f32_in = nc.dram_tensor("f32_in", [P, NF32], f32, kind="ExternalInput").ap()
    y_out = nc.dram_tensor("y_out", [1, P], f32, kind="ExternalOutput").ap()


    with tile.TileContext(nc) as tc, ExitStack() as ctx:
        const = ctx.enter_context(tc.tile_pool(name="const", bufs=1))
        xgp = ctx.enter_context(tc.tile_pool(name="xg", bufs=1))
        colp = ctx.enter_context(tc.tile_pool(name="col", bufs=1))
        spp = ctx.enter_context(tc.tile_pool(name="sp", bufs=len(chunks)))
        scrp = ctx.enter_context(tc.tile_pool(name="scr", bufs=4))
        smallp = ctx.enter_context(tc.tile_pool(name="small", bufs=1))
        epip = ctx.enter_context(tc.tile_pool(name="epi", bufs=1))
        # PSUM banks: accA 1 + accB 1 + uB 1 + tp 3 + outp/outB 2 = 8 (hmm 9)
        accp = ctx.enter_context(tc.tile_pool(name="acc", bufs=1, space="PSUM"))
        ubp = ctx.enter_context(tc.tile_pool(name="ub", bufs=1, space="PSUM"))
        tpp = ctx.enter_context(tc.tile_pool(name="tp", bufs=2, space="PSUM"))
        outpp = ctx.enter_context(tc.tile_pool(name="outp", bufs=1, space="PSUM"))

        # ---- DMAs (f32 consts first: they unblock the setup chain) ----
        f32c = const.tile([P, NF32], f32, tag="f32c")
        nc.sync.dma_start(f32c[:], f32_in[:])
        o = 0
        ident = f32c[:, o : o + P]; o += P
        WTf = f32c[:, o : o + IND]; o += IND
        avs_b = f32c[:, o : o + P]; o += P
        avd_b = f32c[:, o : o + P]; o += P
        bias_b = f32c[:, o : o + OUTD]; o += OUTD
        bias_row = bias_b[0:1, :]
        maskb_t = f32c[:, o : o + TB]
        hdr = xgp.tile([P, XGOFF], bf16, tag="hdr")
        nc.sync.dma_start(hdr[:], xg_in[:, 0:XGOFF])
        xb_t = hdr[:, 0 : TB * W257]
        x0_t = hdr[:, TB * W257 : TB * W257 + IND]
        Wb = hdr[:, TB * W257 + IND : XGOFF]
        dsti_t = colp.tile([P, T], i16, tag="dsti")
        nc.sync.dma_start(dsti_t[:], dsti_in[:])
        dstg_t = colp.tile([P, C2 + T], u16, tag="dstg")
        nc.sync.dma_start(dstg_t[:], dstg_in[:])
        halfmask = dstg_t[:, C2 : C2 + T]
        xg_tiles = []
        off = 0
        for ci, n in enumerate(chunks):
            xc = xgp.tile([P, n * W257], bf16, tag=f"xg{ci}")
            nc.sync.dma_start(
                xc[:], xg_in[:, XGOFF + off * W257 : XGOFF + (off + n) * W257]
            )
            for i in range(n):
                xg_tiles.append((xc, i))
            off += n

        ones_row = const.tile([1, P], f32, tag="ones_row")
        nc.vector.memset(ones_row[:], 1.0)

        # ---- setup: wsb/wdb = broadcast(W @ att) via one matmul each ----
        def att_bcast(att_b, name):
            bp = tpp.tile([P, IND], f32, tag="tp")
            nc.tensor.matmul(bp[:], lhsT=att_b, rhs=WTf[:], start=True, stop=True)
            bs = const.tile([P, IND], bf16, tag=f"{name}_b")
            nc.scalar.copy(bs[:], bp[:])
            return bs

        wdb = att_bcast(avd_b, "wdb")
        wsb = att_bcast(avs_b, "wsb")

        # ---- a_d of this core's dst rows (from the x0 tile) ----
        adM = smallp.tile([P, 1], f32, tag="adM")
        scr0 = scrp.tile([P, IND], bf16, tag="scr")
        nc.vector.scalar_tensor_tensor(
            out=scr0[:], in0=x0_t[:], scalar=0.0, in1=wdb[:],
            op0=Alu.bypass, op1=Alu.mult, accum_out=adM[:],
        )
        adrp = tpp.tile([P, P], f32, tag="tp")
        nc.tensor.transpose(adrp[:1, :], adM[:], ident[:])
        adr = const.tile([1, P], f32, tag="adr")
        nc.scalar.copy(adr[:], adrp[:1, :P])
        adbp = tpp.tile([P, P], f32, tag="tp")
        nc.tensor.matmul(adbp[:], lhsT=ones_row[:], rhs=adr[:], start=True, stop=True)
        adb_b = const.tile([P, P], f32, tag="adb_b")
        nc.scalar.copy(adb_b[:], adbp[:])
        # gather ad2[e, 2t+h] = a_d[dst of (group(e), tile t, half h)]
        ad2 = colp.tile([P, 2 * T], f32, tag="ad2")
        nc.gpsimd.indirect_copy(
            ad2[:], adb_b[:], dstg_t[:, 0:C2], i_know_ap_gather_is_preferred=True
        )
        # per-row half select is emitted later (inside the main loop) so the
        # DVE queue never stalls waiting for the ad2 gather
        ad_all = colp.tile([P, T], f32, tag="ad_all")

        # ---- block B: dst==0 edges, no one-hot needed; epilogue early ----
        uB = ubp.tile([1, W257], f32, tag="uB")
        a_sB = smallp.tile([P, TB], f32, tag="a_sB")
        adBc = smallp.tile([P, TB], f32, tag="adBc")
        for t in range(TB):
            xbt = xb_t[:, t * W257 : t * W257 + IND]
            scrb = scrp.tile([P, IND], bf16, tag="scr")
            nc.vector.scalar_tensor_tensor(
                out=scrb[:], in0=xbt, scalar=0.0, in1=wsb[:],
                op0=Alu.bypass, op1=Alu.mult, accum_out=a_sB[:, t : t + 1],
            )
            scrb2 = scrp.tile([P, IND], bf16, tag="scr")
            nc.vector.scalar_tensor_tensor(
                out=scrb2[:], in0=xbt, scalar=0.0, in1=wdb[:],
                op0=Alu.bypass, op1=Alu.mult, accum_out=adBc[:, t : t + 1],
            )
        # ad0 = adBc[0,0] (self-loop of node 0 in slot 0) broadcast to [P,1]
        ad0p = tpp.tile([P, P], f32, tag="tp")
        nc.tensor.matmul(
            ad0p[:, 0:1], lhsT=ones_row[:], rhs=adBc[0:1, 0:1], start=True, stop=True
        )
        ad0 = smallp.tile([P, 1], f32, tag="ad0")
        nc.scalar.copy(ad0[:], ad0p[:, 0:1])
        for t in range(TB):
            vB = smallp.tile([P, 1], f32, tag="vB")
            nc.vector.tensor_tensor(
                out=vB[:], in0=a_sB[:, t : t + 1], in1=ad0[:], op=Alu.add
            )
            eB = smallp.tile([P, 1], f32, tag="eB")
            nc.vector.scalar_tensor_tensor(
                out=eB[:], in0=vB[:], scalar=NEG_SLOPE, in1=vB[:],
                op0=Alu.mult, op1=Alu.max,
            )
            pB = smallp.tile([P, 1], f32, tag="pB")
            nc.scalar.activation(pB[:], eB[:], Act.Exp)
            pBm = smallp.tile([P, 1], bf16, tag="pBm")
            nc.scalar.activation(
                pBm[:], pB[:], Act.Copy, scale=maskb_t[:, t : t + 1]
            )
            nc.tensor.matmul(
                uB[:], lhsT=pBm[:], rhs=xb_t[:, t * W257 : (t + 1) * W257],
                start=(t == 0), stop=(t == TB - 1), skip_group_check=True,
            )
        xo0s = epip.tile([P, OUTD], f32, tag="xo0s")

        def emit_b_epilogue():
            uB_sb = epip.tile([1, W257], f32, tag="uB_sb")
            nc.scalar.copy(uB_sb[:], uB[:])
            outB = outpp.tile([1, OUTD], f32, tag="outB")
            for ci in range(2):
                ubtp = tpp.tile([P, P], f32, tag="tp")
                nc.tensor.transpose(
                    ubtp[:, :1], uB_sb[:1, ci * P : (ci + 1) * P], ident[0:1, 0:1]
                )
                uBT = smallp.tile([P, 1], bf16, tag=f"uBT{ci}")
                nc.scalar.copy(uBT[:], ubtp[:, :1])
                nc.tensor.matmul(
                    outB[:], lhsT=uBT[:], rhs=Wb[:, ci * OUTD : (ci + 1) * OUTD],
                    start=(ci == 0), stop=(ci == 1), skip_group_check=True,
                )
            rzB = smallp.tile([1, 1], f32, tag="rzB")
            nc.vector.reciprocal(rzB[:], uB_sb[:1, IND : IND + 1])
            outnB = epip.tile([1, OUTD], f32, tag="outnB")
            nc.vector.scalar_tensor_tensor(
                out=outnB[:], in0=outB[:], scalar=rzB[:], in1=bias_row[:],
                op0=Alu.mult, op1=Alu.add,
            )
            # elu(x) = max(exp(-relu(-x)) - 1, x)
            tposB = smallp.tile([1, OUTD], f32, tag="tposB")
            nc.scalar.activation(tposB[:], outnB[:], Act.Relu, scale=-1.0)
            texpB = smallp.tile([1, OUTD], f32, tag="texpB")
            nc.scalar.activation(texpB[:], tposB[:], Act.Exp, scale=-1.0)
            xoB = epip.tile([1, OUTD], f32, tag="xoB")
            nc.vector.scalar_tensor_tensor(
                out=xoB[:], in0=texpB[:], scalar=-1.0, in1=outnB[:],
                op0=Alu.add, op1=Alu.max,
            )
            xo0p = tpp.tile([P, P], f32, tag="tp")
            nc.tensor.matmul(
                xo0p[:, :OUTD], lhsT=ones_row[:], rhs=xoB[:], start=True, stop=True
            )
            nc.scalar.copy(xo0s[:], xo0p[:, :OUTD])

        # ---- main pipeline ----
        acc = accp.tile([P, W257], f32, tag="acc")
        bounds = []
        s = 0
        for n in chunks:
            bounds.append((s, s + n))
            s += n

        # DVE queue discipline: a_s of chunk c, then (select / B-epilogue /
        # v,e of chunk c-1) whose inputs are ready by the time DVE gets there
        def emit_as(ci):
            lo, hi = bounds[ci]
            as_t = smallp.tile([P, hi - lo], f32, tag=f"as{ci}")
            for t in range(lo, hi):
                xc, i = xg_tiles[t]
                scr = scrp.tile([P, IND], bf16, tag="scr")
                nc.vector.scalar_tensor_tensor(
                    out=scr[:], in0=xc[:, i * W257 : i * W257 + IND], scalar=0.0,
                    in1=wsb[:], op0=Alu.bypass, op1=Alu.mult,
                    accum_out=as_t[:, t - lo : t - lo + 1],
                )
            return as_t

        def emit_rest(ci, as_t):
            lo, hi = bounds[ci]
            w = hi - lo
            v_t = smallp.tile([P, w], f32, tag=f"v{ci}")
            nc.vector.tensor_tensor(
                out=v_t[:], in0=as_t[:], in1=ad_all[:, lo:hi], op=Alu.add
            )
            e_t = smallp.tile([P, w], f32, tag=f"e{ci}")
            nc.vector.scalar_tensor_tensor(
                out=e_t[:], in0=v_t[:], scalar=NEG_SLOPE, in1=v_t[:],
                op0=Alu.mult, op1=Alu.max,
            )
            p_t = smallp.tile([P, w], bf16, tag=f"p{ci}")
            nc.scalar.activation(p_t[:], e_t[:], Act.Exp)
            # canary: a standard-opcode gpsimd op absorbs the wait for p_t
            # cheaply; a local_scatter that waits at arrival can burn ~7us
            cang = smallp.tile([P, 1], bf16, tag=f"can{ci}")
            nc.gpsimd.tensor_tensor(
                out=cang[:], in0=p_t[:, w - 1 : w], in1=p_t[:, w - 1 : w],
                op=Alu.add,
            )
            Sp = spp.tile([P, w * P], bf16, tag="Sp")
            nc.gpsimd.local_scatter(
                Sp[:], p_t[:], dsti_t[:, lo:hi],
                channels=P, num_elems=w * P, num_idxs=w,
            )
            for t in range(lo, hi):
                xc, i = xg_tiles[t]
                nc.tensor.matmul(
                    acc[:], lhsT=Sp[:, (t - lo) * P : (t - lo + 1) * P],
                    rhs=xc[:, i * W257 : (i + 1) * W257],
                    start=(t == 0), stop=(t == T - 1), skip_group_check=True,
                )

        as_tiles = {}
        for ci in range(len(bounds)):
            as_tiles[ci] = emit_as(ci)
            if ci == 1:
                # ad2 gather + B-block results are ready by now
                nc.vector.select(
                    ad_all[:], halfmask, ad2[:, 1 : 2 * T : 2],
                    ad2[:, 0 : 2 * T : 2],
                )
                emit_b_epilogue()
                emit_rest(0, as_tiles[0])
            elif ci >= 2:
                emit_rest(ci - 1, as_tiles[ci - 1])
        emit_rest(len(bounds) - 1, as_tiles[len(bounds) - 1])

        # ---- epilogue: out = (u@W)/z + bias; xo = elu(out) ----
        acc_sb = epip.tile([P, W257], f32, tag="acc_sb")
        nc.scalar.copy(acc_sb[:], acc[:])
        outp = outpp.tile([P, OUTD], f32, tag="outp")
        for ci in range(2):
            utp = tpp.tile([P, P], f32, tag="tp")
            nc.tensor.transpose(utp[:], acc_sb[:, ci * P : (ci + 1) * P], ident[:])
            u = epip.tile([P, P], bf16, tag=f"uT{ci}")
            if ci == 0:
                nc.scalar.copy(u[:], utp[:])
            else:
                nc.vector.tensor_copy(u[:], utp[:])
            nc.tensor.matmul(
                outp[:], lhsT=u[:], rhs=Wb[:, ci * OUTD : (ci + 1) * OUTD],
                start=(ci == 0), stop=(ci == 1), skip_group_check=True,
            )
        rz = smallp.tile([P, 1], f32, tag="rz")
        nc.vector.reciprocal(rz[:], acc_sb[:, IND : IND + 1])
        outn = epip.tile([P, OUTD], f32, tag="outn")
        nc.vector.scalar_tensor_tensor(
            out=outn[:], in0=outp[:], scalar=rz[:], in1=bias_b[:],
            op0=Alu.mult, op1=Alu.add,
        )
        tpos = epip.tile([P, OUTD], f32, tag="tpos")
        nc.scalar.activation(tpos[:], outn[:], Act.Relu, scale=-1.0)
        texp = epip.tile([P, OUTD], f32, tag="texp")
        nc.scalar.activation(texp[:], tpos[:], Act.Exp, scale=-1.0)
        xo = epip.tile([P, OUTD], f32, tag="xo")
        nc.vector.scalar_tensor_tensor(
            out=xo[:], in0=texp[:], scalar=-1.0, in1=outn[:],
            op0=Alu.add, op1=Alu.max,
        )

        # ---- y = elu(dot(xo0, xo_j)) ----
        dscr = scrp.tile([P, OUTD], bf16, tag="dscr")
        d_sb = smallp.tile([P, 1], f32, tag="d")
        nc.vector.scalar_tensor_tensor(
            out=dscr[:], in0=xo[:], scalar=0.0, in1=xo0s[:],
            op0=Alu.bypass, op1=Alu.mult, accum_out=d_sb[:],
        )
        ypos = smallp.tile([P, 1], f32, tag="ypos")
        nc.scalar.activation(ypos[:], d_sb[:], Act.Relu, scale=-1.0)
        yexp = smallp.tile([P, 1], f32, tag="yexp")
        nc.scalar.activation(yexp[:], ypos[:], Act.Exp, scale=-1.0)
        y_sb = smallp.tile([P, 1], f32, tag="y_sb")
        nc.vector.scalar_tensor_tensor(
            out=y_sb[:], in0=yexp[:], scalar=-1.0, in1=d_sb[:],
            op0=Alu.add, op1=Alu.max,
        )
        yrp = tpp.tile([P, P], f32, tag="tp")
        nc.tensor.transpose(yrp[:1, :], y_sb[:], ident[:])
        y_row = epip.tile([1, P], f32, tag="y_row")
        nc.scalar.copy(y_row[:], yrp[:1, :P])
        nc.scalar.dma_start(y_out[:], y_row[:])

    nc.compile()
    return nc


def _get_program(T, TB, chunks):
    key = (T, TB, tuple(chunks))
    if key not in _CACHE:
        _CACHE[key] = _build_program(T, TB, chunks)
    return _CACHE[key]


def prepare(x, edge_index, W, att_src, att_dst, bias, item_len):
    """Host-side edge partitioning + feature gather; returns (nc, in_maps, item_len)."""
    import ml_dtypes

    bf16 = ml_dtypes.bfloat16
    item_len = int(np.asarray(item_len))
    x = np.ascontiguousarray(np.asarray(x, np.float32))
    W = np.ascontiguousarray(np.asarray(W, np.float32))
    att_src = np.asarray(att_src, np.float32)
    att_dst = np.asarray(att_dst, np.float32)
    bias = np.asarray(bias, np.float32)
    n_nodes, in_dim = x.shape
    out_dim = W.shape[1]
    assert in_dim == IND and out_dim == OUTD
    assert item_len <= N_CORES * P, "kernel supports item_len <= 1024"

    src = np.asarray(edge_index[0])
    dst = np.asarray(edge_index[1])
    keep = dst < item_len
    src_f = src[keep].astype(np.int32)
    dst_f = dst[keep].astype(np.int32)

    # sort edges by dst; append self-loops so every dst row has >= 1 edge
    loops = np.arange(item_len, dtype=np.int32)
    src_f = np.concatenate([src_f, loops])
    dst_f = np.concatenate([dst_f, loops])
    order = np.argsort(dst_f, kind="stable")
    src_f = src_f[order]
    dst_f = dst_f[order]
    row_start = np.searchsorted(dst_f, np.arange(item_len + 1))

    # per core: 8-edge same-dst sub-groups; 16 sub-slots of 8 rows per tile
    core_glists = []
    Gmax = 0
    for k in range(N_CORES):
        glist = []  # (dst_local, srcs[<=8])
        for j in range(P):
            row = k * P + j
            if row >= item_len:
                continue
            lo, hi = row_start[row], row_start[row + 1]
            for s in range(lo, hi, 8):
                glist.append((j, src_f[s : min(s + 8, hi)]))
        core_glists.append(glist)
        Gmax = max(Gmax, len(glist))
    T = int(math.ceil(Gmax / 16))
    if T % 2:
        T += 1
    chunks = _chunk_sizes(T)

    # block B: dst == 0 edges incl the (0,0) self-loop, loop moved to slot 0
    b_all = src_f[row_start[0] : row_start[1]]  # graph edges first, loop last
    b_src = np.concatenate([[0], b_all[:-1]]).astype(np.int32)
    nB = len(b_src)
    TB = max(1, int(math.ceil(nB / P)))
    b_pad = TB * P - nB
    b_src = np.concatenate([b_src, np.zeros(b_pad, np.int32)])
    maskB = np.concatenate([np.ones(nB, np.float32), np.zeros(b_pad, np.float32)])
    xbg = x[b_src]
    xb_pack = np.concatenate([xbg, np.ones((TB * P, 1), np.float32)], axis=1)
    xb_pack = (
        xb_pack.reshape(TB, P, IND + 1).transpose(1, 0, 2).reshape(P, TB * (IND + 1))
    )
    xb_bf = xb_pack.astype(bf16)
    maskB = maskB.reshape(TB, P).T

    nc = _get_program(T, TB, chunks)

    # chunk-local scatter index offsets
    tile_off = np.zeros(T, np.int32)
    s = 0
    for n in chunks:
        for i in range(n):
            tile_off[s + i] = i * P
        s += n

    W_bf = np.concatenate([W[0:P, :], W[P : 2 * P, :]], axis=1).astype(bf16)
    # f32 pack: ident | WT | avs_b | avd_b | bias_b | halfmask | maskB
    f32_pack = np.ascontiguousarray(
        np.concatenate(
            [
                np.eye(P, dtype=np.float32),
                W.T.astype(np.float32),
                np.tile(att_src[:, None], (1, P)),
                np.tile(att_dst[:, None], (1, P)),
                np.tile(bias.reshape(1, out_dim), (P, 1)),
                maskB,
            ],
            axis=1,
        )
    )
    halfmask_u16 = np.tile(
        ((np.arange(P) % 16) >= 8).astype(np.uint16)[:, None], (1, T)
    )

    C2 = (2 * T + 15) // 16
    in_maps = []
    for k in range(N_CORES):
        glist = core_glists[k]
        src_slot = np.zeros((P, T), np.int32)
        dst_slot = np.full((P, T), -1, np.int32)  # -1 = pad
        D2 = np.zeros((8, 2 * T), np.int32)  # dst per (group, tile, half)
        for gi, (j, srcs) in enumerate(glist):
            t = gi // 16
            sub = gi % 16  # sub-slot: group g = sub // 2, half h = sub % 2
            g, h = sub // 2, sub % 2
            D2[g, 2 * t + h] = j
            rows = 16 * g + 8 * h + np.arange(len(srcs))
            src_slot[rows, t] = srcs
            dst_slot[rows, t] = j
        xg = x[src_slot.T.reshape(-1)]  # [T*P, IND] tile-major
        xg_pack = np.concatenate([xg, np.ones((T * P, 1), np.float32)], axis=1)
        xg_pack = (
            xg_pack.reshape(T, P, IND + 1).transpose(1, 0, 2).reshape(P, T * (IND + 1))
        )
        xg_bf = xg_pack.astype(bf16)
        dsti = np.where(dst_slot >= 0, dst_slot + tile_off[None, :], -1).astype(
            np.int16
        )
        dstg = np.zeros((P, C2), np.uint16)
        for g in range(8):
            for i in range(2 * T):
                dstg[16 * g + (i % 16), i // 16] = D2[g, i]
        dstg = np.concatenate([dstg, halfmask_u16], axis=1)
        mrows = np.minimum(np.arange(k * P, (k + 1) * P, dtype=np.int32), n_nodes - 1)
        x0_bf = x[mrows].astype(bf16)
        # bf16 pack: [xb tiles | x0 | W halves | xg tiles]
        xg_full = np.ascontiguousarray(
            np.concatenate([xb_bf, x0_bf, W_bf, xg_bf], axis=1)
        )
        in_maps.append(
            {
                "xg_in": xg_full,
                "dsti_in": np.ascontiguousarray(dsti),
                "dstg_in": np.ascontiguousarray(dstg),
                "f32_in": f32_pack,
            }
        )
    return nc, in_maps, item_len


def assemble(results, item_len):
    y_all = np.concatenate([results[k]["y_out"].ravel() for k in range(N_CORES)])
    return y_all[1:item_len].astype(np.float32)


def kernel(x, edge_index, W, att_src, att_dst, bias, item_len):
    from concourse import bass_utils

    nc, in_maps, item_len = prepare(
        x, edge_index, W, att_src, att_dst, bias, item_len
    )
    res = bass_utils.run_bass_kernel_spmd(nc, in_maps, core_ids=list(range(N_CORES)))
    return assemble(res.results, item_len)


# revision 45
# speedup vs baseline: 2.1250x; 2.1250x over previous
"""GAT message-passing kernel for Trainium2 (8 NeuronCores, Bass/Tile).

Strategy v6 (edge-parallel, host-packed gather, gpsimd one-hot scatter):
y = elu(sum(xo[0] * xo[1:item_len], 1)) depends only on output rows
0..item_len-1, so only edges with dst < item_len contribute (~33.7k of
3.2M).  The host filters those edges, partitions them by dst block of
128 (core k owns dst rows [128k, 128k+128)), gathers x[src] rows and
packs them bf16 as [128 edges/tile | 256 feats + ones column] tiles.
Edges are arranged so every 8-partition half of a 16-row gpsimd group
shares one dst (~10% padding): ONE gpsimd indirect_copy (whose indices
are shared per 16-partition group) fetches a_d[dst] for both halves at
2 columns/tile, and a DVE select picks each row's half.

  per chunk of tiles (pipelined across engines):
    DVE:    a_s[:,t] = rowsum(xg_t * (W@att_src))        (stt accum)
    DVE:    v = a_s + ad; e = lrelu(v)                   (batched)
    Act:    p = exp(e)  -> bf16                          (batched)
    GpSimd: Sp[e, 128*i + dst] = p[e,i]  (local_scatter, one op/chunk;
            pad edges use index -1 and are dropped)
    PE:     acc += Sp_i^T @ [xg_i | 1]   (PSUM accumulate, bf16)
  acc is split in two PSUM halves so the first half's (u@W) epilogue
  overlaps the second half of the pipeline; u@W uses PE transposes.
  out = (u@W)/z + bias; xo = elu(out); y_k[j] = elu(dot(xo0, xo_j)).
  Every core also processes the dst==0 edges (block B, no one-hot:
  uB = p^T @ [xg|1], epilogue emitted early) so xo0 is ready before
  the main tail; Python concatenates the 8 y_k slices.
"""
import math

import numpy as np

P = 128
N_CORES = 8
NEG_SLOPE = 0.2
IND = 256
OUTD = 128

_CACHE = {}


def _chunk_sizes(T):
    # even-sized chunks; small first (fast pipeline start), small last
    sizes = [2]
    rem = T - 2
    while rem > 10:
        sizes.append(8)
        rem -= 8
    if rem > 2:
        sizes.append(rem - 2)
        sizes.append(2)
    elif rem > 0:
        sizes.append(rem)
    return sizes


def _build_program(T, TB, chunks):
    import concourse.bass as bass
    import concourse.bacc as bacc
    import concourse.tile as tile
    import concourse.mybir as mybir
    from contextlib import ExitStack

    f32 = mybir.dt.float32
    bf16 = mybir.dt.bfloat16
    i16 = mybir.dt.int16
    u16 = mybir.dt.uint16
    Alu = mybir.AluOpType
    Act = mybir.ActivationFunctionType
    W257 = IND + 1

    nc = bacc.Bacc(
        "TRN2", target_bir_lowering=False, debug=False, num_devices=N_CORES
    )
    C2 = (2 * T + 15) // 16
    # bf16 pack: [xb tiles | x0 | W halves | xg tiles]
    XGOFF = TB * W257 + IND + 2 * OUTD
    xg_in = nc.dram_tensor(
        "xg_in", [P, XGOFF + T * W257], bf16, kind="ExternalInput"
    ).ap()
    # scatter idx (chunk-local offsets, -1 pads) | gather idx (group-wrapped)
    dsti_in = nc.dram_tensor("dsti_in", [P, T], i16, kind="ExternalInput").ap()
    # gather idx | uint16 halfmask (rows 8..15 of each 16-group = 1)
    dstg_in = nc.dram_tensor(
        "dstg_in", [P, C2 + T], u16, kind="ExternalInput"
    ).ap()
    # f32 pack: ident(P) | WT(IND) | avs_b(P) | avd_b(P) | bias_b(OUTD) | maskB(TB)
    NF32 = P + IND + P + P + OUTD + TB
    f32_in = nc.dram_tensor("f32_in", [P, NF32], f32, kind="ExternalInput").ap()
    y_out = nc.dram_tensor("y_out", [1, P], f32, kind="ExternalOutput").ap()


    with tile.TileContext(nc) as tc, ExitStack() as ctx:
        const = ctx.enter_context(tc.tile_pool(name="const", bufs=1))
        xgp = ctx.enter_context(tc.tile_pool(name="xg", bufs=1))
        colp = ctx.enter_context(tc.tile_pool(name="col", bufs=1))
        spp = ctx.enter_context(tc.tile_pool(name="sp", bufs=len(chunks)))
        scrp = ctx.enter_context(tc.tile_pool(name="scr", bufs=4))
        smallp = ctx.enter_context(tc.tile_pool(name="small", bufs=1))
        epip = ctx.enter_context(tc.tile_pool(name="epi", bufs=1))
        # PSUM banks: accA 1 + accB 1 + uB 1 + tp 3 + outp/outB 2 = 8 (hmm 9)
        accp = ctx.enter_context(tc.tile_pool(name="acc", bufs=1, space="PSUM"))
        ubp = ctx.enter_context(tc.tile_pool(name="ub", bufs=1, space="PSUM"))
        tpp = ctx.enter_context(tc.tile_pool(name="tp", bufs=2, space="PSUM"))
        outpp = ctx.enter_context(tc.tile_pool(name="outp", bufs=1, space="PSUM"))

        # ---- DMAs (f32 consts first: they unblock the setup chain) ----
        f32c = const.tile([P, NF32], f32, tag="f32c")
        nc.sync.dma_start(f32c[:], f32_in[:])
        o = 0
        ident = f32c[:, o : o + P]; o += P
        WTf = f32c[:, o : o + IND]; o += IND
        avs_b = f32c[:, o : o + P]; o += P
        avd_b = f32c[:, o : o + P]; o += P
        bias_b = f32c[:, o : o + OUTD]; o += OUTD
        bias_row = bias_b[0:1, :]
        maskb_t = f32c[:, o : o + TB]
        hdr = xgp.tile([P, XGOFF], bf16, tag="hdr")
        nc.sync.dma_start(hdr[:], xg_in[:, 0:XGOFF])
        xb_t = hdr[:, 0 : TB * W257]
        x0_t = hdr[:, TB * W257 : TB * W257 + IND]
        Wb = hdr[:, TB * W257 + IND : XGOFF]
        dsti_t = colp.tile([P, T], i16, tag="dsti")
        nc.sync.dma_start(dsti_t[:], dsti_in[:])
        dstg_t = colp.tile([P, C2 + T], u16, tag="dstg")
        nc.sync.dma_start(dstg_t[:], dstg_in[:])
        halfmask = dstg_t[:, C2 : C2 + T]
        xg_tiles = []
        off = 0
        for ci, n in enumerate(chunks):
            xc = xgp.tile([P, n * W257], bf16, tag=f"xg{ci}")
            nc.sync.dma_start(
                xc[:], xg_in[:, XGOFF + off * W257 : XGOFF + (off + n) * W257]
            )
            for i in range(n):
                xg_tiles.append((xc, i))
            off += n

        ones_row = const.tile([1, P], f32, tag="ones_row")
        nc.vector.memset(ones_row[:], 1.0)

        # ---- setup: wsb/wdb = broadcast(W @ att) via one matmul each ----
        def att_bcast(att_b, name):
            bp = tpp.tile([P, IND], f32, tag="tp")
            nc.tensor.matmul(bp[:], lhsT=att_b, rhs=WTf[:], start=True, stop=True)
            bs = const.tile([P, IND], bf16, tag=f"{name}_b")
            nc.scalar.copy(bs[:], bp[:])
            return bs

        wdb = att_bcast(avd_b, "wdb")
        wsb = att_bcast(avs_b, "wsb")

        # ---- a_d of this core's dst rows (from the x0 tile) ----
        adM = smallp.tile([P, 1], f32, tag="adM")
        scr0 = scrp.tile([P, IND], bf16, tag="scr")
        nc.vector.scalar_tensor_tensor(
            out=scr0[:], in0=x0_t[:], scalar=0.0, in1=wdb[:],
            op0=Alu.bypass, op1=Alu.mult, accum_out=adM[:],
        )
        adrp = tpp.tile([P, P], f32, tag="tp")
        nc.tensor.transpose(adrp[:1, :], adM[:], ident[:])
        adr = const.tile([1, P], f32, tag="adr")
        nc.scalar.copy(adr[:], adrp[:1, :P])
        adbp = tpp.tile([P, P], f32, tag="tp")
        nc.tensor.matmul(adbp[:], lhsT=ones_row[:], rhs=adr[:], start=True, stop=True)
        adb_b = const.tile([P, P], f32, tag="adb_b")
        nc.scalar.copy(adb_b[:], adbp[:])
        # gather ad2[e, 2t+h] = a_d[dst of (group(e), tile t, half h)]
        ad2 = colp.tile([P, 2 * T], f32, tag="ad2")
        nc.gpsimd.indirect_copy(
            ad2[:], adb_b[:], dstg_t[:, 0:C2], i_know_ap_gather_is_preferred=True
        )
        # per-row half select is emitted later (inside the main loop) so the
        # DVE queue never stalls waiting for the ad2 gather
        ad_all = colp.tile([P, T], f32, tag="ad_all")

        # ---- block B: dst==0 edges, no one-hot needed; epilogue early ----
        uB = ubp.tile([1, W257], f32, tag="uB")
        a_sB = smallp.tile([P, TB], f32, tag="a_sB")
        adBc = smallp.tile([P, TB], f32, tag="adBc")
        for t in range(TB):
            xbt = xb_t[:, t * W257 : t * W257 + IND]
            scrb = scrp.tile([P, IND], bf16, tag="scr")
            nc.vector.scalar_tensor_tensor(
                out=scrb[:], in0=xbt, scalar=0.0, in1=wsb[:],
                op0=Alu.bypass, op1=Alu.mult, accum_out=a_sB[:, t : t + 1],
            )
            scrb2 = scrp.tile([P, IND], bf16, tag="scr")
            nc.vector.scalar_tensor_tensor(
                out=scrb2[:], in0=xbt, scalar=0.0, in1=wdb[:],
                op0=Alu.bypass, op1=Alu.mult, accum_out=adBc[:, t : t + 1],
            )
        # ad0 = adBc[0,0] (self-loop of node 0 in slot 0) broadcast to [P,1]
        ad0p = tpp.tile([P, P], f32, tag="tp")
        nc.tensor.matmul(
            ad0p[:, 0:1], lhsT=ones_row[:], rhs=adBc[0:1, 0:1], start=True, stop=True
        )
        ad0 = smallp.tile([P, 1], f32, tag="ad0")
        nc.scalar.copy(ad0[:], ad0p[:, 0:1])
        for t in range(TB):
            vB = smallp.tile([P, 1], f32, tag="vB")
            nc.vector.tensor_tensor(
                out=vB[:], in0=a_sB[:, t : t + 1], in1=ad0[:], op=Alu.add
            )
            eB = smallp.tile([P, 1], f32, tag="eB")
            nc.vector.scalar_tensor_tensor(
                out=eB[:], in0=vB[:], scalar=NEG_SLOPE, in1=vB[:],
                op0=Alu.mult, op1=Alu.max,
            )
            pB = smallp.tile([P, 1], f32, tag="pB")
            nc.scalar.activation(pB[:], eB[:], Act.Exp)
            pBm = smallp.tile([P, 1], bf16, tag="pBm")
            nc.scalar.activation(
                pBm[:], pB[:], Act.Copy, scale=maskb_t[:, t : t + 1]
            )
            nc.tensor.matmul(
                uB[:], lhsT=pBm[:], rhs=xb_t[:, t * W257 : (t + 1) * W257],
                start=(t == 0), stop=(t == TB - 1), skip_group_check=True,
            )
        xo0s = epip.tile([P, OUTD], f32, tag="xo0s")

        def emit_b_epilogue():
            uB_sb = epip.tile([1, W257], f32, tag="uB_sb")
            nc.scalar.copy(uB_sb[:], uB[:])
            outB = outpp.tile([1, OUTD], f32, tag="outB")
            for ci in range(2):
                ubtp = tpp.tile([P, P], f32, tag="tp")
                nc.tensor.transpose(
                    ubtp[:, :1], uB_sb[:1, ci * P : (ci + 1) * P], ident[0:1, 0:1]
                )
                uBT = smallp.tile([P, 1], bf16, tag=f"uBT{ci}")
                nc.scalar.copy(uBT[:], ubtp[:, :1])
                nc.tensor.matmul(
                    outB[:], lhsT=uBT[:], rhs=Wb[:, ci * OUTD : (ci + 1) * OUTD],
                    start=(ci == 0), stop=(ci == 1), skip_group_check=True,
                )
            rzB = smallp.tile([1, 1], f32, tag="rzB")
            nc.vector.reciprocal(rzB[:], uB_sb[:1, IND : IND + 1])
            outnB = epip.tile([1, OUTD], f32, tag="outnB")
            nc.vector.scalar_tensor_tensor(
                out=outnB[:], in0=outB[:], scalar=rzB[:], in1=bias_row[:],
                op0=Alu.mult, op1=Alu.add,
            )
            # elu(x) = max(exp(-relu(-x)) - 1, x)
            tposB = smallp.tile([1, OUTD], f32, tag="tposB")
            nc.scalar.activation(tposB[:], outnB[:], Act.Relu, scale=-1.0)
            texpB = smallp.tile([1, OUTD], f32, tag="texpB")
            nc.scalar.activation(texpB[:], tposB[:], Act.Exp, scale=-1.0)
            xoB = epip.tile([1, OUTD], f32, tag="xoB")
            nc.vector.scalar_tensor_tensor(
                out=xoB[:], in0=texpB[:], scalar=-1.0, in1=outnB[:],
                op0=Alu.add, op1=Alu.max,
            )
            xo0p = tpp.tile([P, P], f32, tag="tp")
            nc.tensor.matmul(
                xo0p[:, :OUTD], lhsT=ones_row[:], rhs=xoB[:], start=True, stop=True
            )
            nc.scalar.copy(xo0s[:], xo0p[:, :OUTD])

        # ---- main pipeline ----
        acc = accp.tile([P, W257], f32, tag="acc")
        bounds = []
        s = 0
        for n in chunks:
            bounds.append((s, s + n))
            s += n

        # DVE queue discipline: a_s of chunk c, then (select / B-epilogue /
        # v,e of chunk c-1) whose inputs are ready by the time DVE gets there
        def emit_as(ci):
            lo, hi = bounds[ci]
            as_t = smallp.tile([P, hi - lo], f32, tag=f"as{ci}")
            for t in range(lo, hi):
                xc, i = xg_tiles[t]
                scr = scrp.tile([P, IND], bf16, tag="scr")
                nc.vector.scalar_tensor_tensor(
                    out=scr[:], in0=xc[:, i * W257 : i * W257 + IND], scalar=0.0,
                    in1=wsb[:], op0=Alu.bypass, op1=Alu.mult,
                    accum_out=as_t[:, t - lo : t - lo + 1],
                )
            return as_t

        def emit_rest(ci, as_t):
            lo, hi = bounds[ci]
            w = hi - lo
            v_t = smallp.tile([P, w], f32, tag=f"v{ci}")
            nc.vector.tensor_tensor(
                out=v_t[:], in0=as_t[:], in1=ad_all[:, lo:hi], op=Alu.add
            )
            e_t = smallp.tile([P, w], f32, tag=f"e{ci}")
            nc.vector.scalar_tensor_tensor(
                out=e_t[:], in0=v_t[:], scalar=NEG_SLOPE, in1=v_t[:],
                op0=Alu.mult, op1=Alu.max,
            )
            p_t = smallp.tile([P, w], bf16, tag=f"p{ci}")
            nc.scalar.activation(p_t[:], e_t[:], Act.Exp)
            Sp = spp.tile([P, w * P], bf16, tag="Sp")
            nc.gpsimd.local_scatter(
                Sp[:], p_t[:], dsti_t[:, lo:hi],
                channels=P, num_elems=w * P, num_idxs=w,
            )
            for t in range(lo, hi):
                xc, i = xg_tiles[t]
                nc.tensor.matmul(
                    acc[:], lhsT=Sp[:, (t - lo) * P : (t - lo + 1) * P],
                    rhs=xc[:, i * W257 : (i + 1) * W257],
                    start=(t == 0), stop=(t == T - 1), skip_group_check=True,
                )

        as_tiles = {}
        for ci in range(len(bounds)):
            as_tiles[ci] = emit_as(ci)
            if ci == 1:
                # ad2 gather + B-block results are ready by now
                nc.vector.select(
                    ad_all[:], halfmask, ad2[:, 1 : 2 * T : 2],
                    ad2[:, 0 : 2 * T : 2],
                )
                emit_b_epilogue()
                emit_rest(0, as_tiles[0])
            elif ci >= 2:
                emit_rest(ci - 1, as_tiles[ci - 1])
        emit_rest(len(bounds) - 1, as_tiles[len(bounds) - 1])

        # ---- epilogue: out = (u@W)/z + bias; xo = elu(out) ----
        acc_sb = epip.tile([P, W257], f32, tag="acc_sb")
        nc.scalar.copy(acc_sb[:], acc[:])
        outp = outpp.tile([P, OUTD], f32, tag="outp")
        for ci in range(2):
            utp = tpp.tile([P, P], f32, tag="tp")
            nc.tensor.transpose(utp[:], acc_sb[:, ci * P : (ci + 1) * P], ident[:])
            u = epip.tile([P, P], bf16, tag=f"uT{ci}")
            if ci == 0:
                nc.scalar.copy(u[:], utp[:])
            else:
                nc.vector.tensor_copy(u[:], utp[:])
            nc.tensor.matmul(
                outp[:], lhsT=u[:], rhs=Wb[:, ci * OUTD : (ci + 1) * OUTD],
                start=(ci == 0), stop=(ci == 1), skip_group_check=True,
            )
        rz = smallp.tile([P, 1], f32, tag="rz")
        nc.vector.reciprocal(rz[:], acc_sb[:, IND : IND + 1])
        outn = epip.tile([P, OUTD], f32, tag="outn")
        nc.vector.scalar_tensor_tensor(
            out=outn[:], in0=outp[:], scalar=rz[:], in1=bias_b[:],
            op0=Alu.mult, op1=Alu.add,
        )
        tpos = epip.tile([P, OUTD], f32, tag="tpos")
        nc.scalar.activation(tpos[:], outn[:], Act.Relu, scale=-1.0)
        texp = epip.tile([P, OUTD], f32, tag="texp")
        nc.scalar.activation(texp[:], tpos[:], Act.Exp, scale=-1.0)
        xo = epip.tile([P, OUTD], f32, tag="xo")
        nc.vector.scalar_tensor_tensor(
            out=xo[:], in0=texp[:], scalar=-1.0, in1=outn[:],
            op0=Alu.add, op1=Alu.max,
        )

        # ---- y = elu(dot(xo0, xo_j)) ----
        dscr = scrp.tile([P, OUTD], bf16, tag="dscr")
        d_sb = smallp.tile([P, 1], f32, tag="d")
        nc.vector.scalar_tensor_tensor(
            out=dscr[:], in0=xo[:], scalar=0.0, in1=xo0s[:],
            op0=Alu.bypass, op1=Alu.mult, accum_out=d_sb[:],
        )
        ypos = smallp.tile([P, 1], f32, tag="ypos")
        nc.scalar.activation(ypos[:], d_sb[:], Act.Relu, scale=-1.0)
        yexp = smallp.tile([P, 1], f32, tag="yexp")
        nc.scalar.activation(yexp[:], ypos[:], Act.Exp, scale=-1.0)
        y_sb = smallp.tile([P, 1], f32, tag="y_sb")
        nc.vector.scalar_tensor_tensor(
            out=y_sb[:], in0=yexp[:], scalar=-1.0, in1=d_sb[:],
            op0=Alu.add, op1=Alu.max,
        )
        yrp = tpp.tile([P, P], f32, tag="tp")
        nc.tensor.transpose(yrp[:1, :], y_sb[:], ident[:])
        y_row = epip.tile([1, P], f32, tag="y_row")
        nc.scalar.copy(y_row[:], yrp[:1, :P])
        nc.scalar.dma_start(y_out[:], y_row[:])

    nc.compile()
    return nc


def _get_program(T, TB, chunks):
    key = (T, TB, tuple(chunks))
    if key not in _CACHE:
        _CACHE[key] = _build_program(T, TB, chunks)
    return _CACHE[key]


def prepare(x, edge_index, W, att_src, att_dst, bias, item_len):
    """Host-side edge partitioning + feature gather; returns (nc, in_maps, item_len)."""
    import ml_dtypes

    bf16 = ml_dtypes.bfloat16
    item_len = int(np.asarray(item_len))
    x = np.ascontiguousarray(np.asarray(x, np.float32))
    W = np.ascontiguousarray(np.asarray(W, np.float32))
    att_src = np.asarray(att_src, np.float32)
    att_dst = np.asarray(att_dst, np.float32)
    bias = np.asarray(bias, np.float32)
    n_nodes, in_dim = x.shape
    out_dim = W.shape[1]
    assert in_dim == IND and out_dim == OUTD
    assert item_len <= N_CORES * P, "kernel supports item_len <= 1024"

    src = np.asarray(edge_index[0])
    dst = np.asarray(edge_index[1])
    keep = dst < item_len
    src_f = src[keep].astype(np.int32)
    dst_f = dst[keep].astype(np.int32)

    # sort edges by dst; append self-loops so every dst row has >= 1 edge
    loops = np.arange(item_len, dtype=np.int32)
    src_f = np.concatenate([src_f, loops])
    dst_f = np.concatenate([dst_f, loops])
    order = np.argsort(dst_f, kind="stable")
    src_f = src_f[order]
    dst_f = dst_f[order]
    row_start = np.searchsorted(dst_f, np.arange(item_len + 1))

    # per core: 8-edge same-dst sub-groups; 16 sub-slots of 8 rows per tile
    core_glists = []
    Gmax = 0
    for k in range(N_CORES):
        glist = []  # (dst_local, srcs[<=8])
        for j in range(P):
            row = k * P + j
            if row >= item_len:
                continue
            lo, hi = row_start[row], row_start[row + 1]
            for s in range(lo, hi, 8):
                glist.append((j, src_f[s : min(s + 8, hi)]))
        core_glists.append(glist)
        Gmax = max(Gmax, len(glist))
    T = int(math.ceil(Gmax / 16))
    if T % 2:
        T += 1
    chunks = _chunk_sizes(T)

    # block B: dst == 0 edges incl the (0,0) self-loop, loop moved to slot 0
    b_all = src_f[row_start[0] : row_start[1]]  # graph edges first, loop last
    b_src = np.concatenate([[0], b_all[:-1]]).astype(np.int32)
    nB = len(b_src)
    TB = max(1, int(math.ceil(nB / P)))
    b_pad = TB * P - nB
    b_src = np.concatenate([b_src, np.zeros(b_pad, np.int32)])
    maskB = np.concatenate([np.ones(nB, np.float32), np.zeros(b_pad, np.float32)])
    xbg = x[b_src]
    xb_pack = np.concatenate([xbg, np.ones((TB * P, 1), np.float32)], axis=1)
    xb_pack = (
        xb_pack.reshape(TB, P, IND + 1).transpose(1, 0, 2).reshape(P, TB * (IND + 1))
    )
    xb_bf = xb_pack.astype(bf16)
    maskB = maskB.reshape(TB, P).T

    nc = _get_program(T, TB, chunks)

    # chunk-local scatter index offsets
    tile_off = np.zeros(T, np.int32)
    s = 0
    for n in chunks:
        for i in range(n):
            tile_off[s + i] = i * P
        s += n

    W_bf = np.concatenate([W[0:P, :], W[P : 2 * P, :]], axis=1).astype(bf16)
    # f32 pack: ident | WT | avs_b | avd_b | bias_b | halfmask | maskB
    f32_pack = np.ascontiguousarray(
        np.concatenate(
            [
                np.eye(P, dtype=np.float32),
                W.T.astype(np.float32),
                np.tile(att_src[:, None], (1, P)),
                np.tile(att_dst[:, None], (1, P)),
                np.tile(bias.reshape(1, out_dim), (P, 1)),
                maskB,
            ],
            axis=1,
        )
    )
    halfmask_u16 = np.tile(
        ((np.arange(P) % 16) >= 8).astype(np.uint16)[:, None], (1, T)
    )

    C2 = (2 * T + 15) // 16
    in_maps = []
    for k in range(N_CORES):
        glist = core_glists[k]
        src_slot = np.zeros((P, T), np.int32)
        dst_slot = np.full((P, T), -1, np.int32)  # -1 = pad
        D2 = np.zeros((8, 2 * T), np.int32)  # dst per (group, tile, half)
        for gi, (j, srcs) in enumerate(glist):
            t = gi // 16
            sub = gi % 16  # sub-slot: group g = sub // 2, half h = sub % 2
            g, h = sub // 2, sub % 2
            D2[g, 2 * t + h] = j
            rows = 16 * g + 8 * h + np.arange(len(srcs))
            src_slot[rows, t] = srcs
            dst_slot[rows, t] = j
        xg = x[src_slot.T.reshape(-1)]  # [T*P, IND] tile-major
        xg_pack = np.concatenate([xg, np.ones((T * P, 1), np.float32)], axis=1)
        xg_pack = (
            xg_pack.reshape(T, P, IND + 1).transpose(1, 0, 2).reshape(P, T * (IND + 1))
        )
        xg_bf = xg_pack.astype(bf16)
        dsti = np.where(dst_slot >= 0, dst_slot + tile_off[None, :], -1).astype(
            np.int16
        )
        dstg = np.zeros((P, C2), np.uint16)
        for g in range(8):
            for i in range(2 * T):
                dstg[16 * g + (i % 16), i // 16] = D2[g, i]
        dstg = np.concatenate([dstg, halfmask_u16], axis=1)
        mrows = np.minimum(np.arange(k * P, (k + 1) * P, dtype=np.int32), n_nodes - 1)
        x0_bf = x[mrows].astype(bf16)
        # bf16 pack: [xb tiles | x0 | W halves | xg tiles]
        xg_full = np.ascontiguousarray(
            np.concatenate([xb_bf, x0_bf, W_bf, xg_bf], axis=1)
        )
        in_maps.append(
            {
                "xg_in": xg_full,
                "dsti_in": np.ascontiguousarray(dsti),
                "dstg_in": np.ascontiguousarray(dstg),
                "f32_in": f32_pack,
            }
        )
    return nc, in_maps, item_len


def assemble(results, item_len):
    y_all = np.concatenate([results[k]["y_out"].ravel() for k in range(N_CORES)])
    return y_all[1:item_len].astype(np.float32)


def kernel(x, edge_index, W, att_src, att_dst, bias, item_len):
    from concourse import bass_utils

    nc, in_maps, item_len = prepare(
        x, edge_index, W, att_src, att_dst, bias, item_len
    )
    res = bass_utils.run_bass_kernel_spmd(nc, in_maps, core_ids=list(range(N_CORES)))
    return assemble(res.results, item_len)
